# revision 1
# baseline (speedup 1.0000x reference)
"""Trainium2 Bass kernel for nn_Conv2DRand: batchnorm (training-mode, batch
stats) + 3x3 SAME conv, NHWC, f32.

Full computation:
    mean/var over (N,H,W) per channel; x_bn = (x-mean)*rsqrt(var+eps) + beta
    out = conv2d(x_bn, kernels, SAME, stride 1, NHWC x HWIO -> NHWC)

Sharding: data-parallel over batch across 8 cores (8 images each); batch
statistics via a tiny cross-core AllReduce of [sum, sumsq] per channel.

Key trick: the BN affine transform is folded into the conv so the elementwise
BN pass over the full tensor disappears:
    out = conv(x_pad, K*s) + c
where s = rsqrt(var+eps), x is padded with padval = mean - beta/s (which makes
x_bn's zero-padding exact), and c[co] = sum_{tap,ci} (K*s)[ci,co] *
(beta/s - mean)[ci] restores the additive part uniformly.

Per core pipeline:
  Phase 1: stream x in [128px, 64ch] tiles; one matmul per tile with
           lhsT = x, rhs = [ones | x] accumulating [sums | x^T x] in PSUM.
           Diagonal of x^T x = per-channel sumsq. AllReduce [64,2] stats.
  Phase 2: per image, transpose rows to channel-major via TensorE into a
           padded [64, 114*114] buffer; 3x3 conv = 9 accumulating matmuls
           (lhsT = folded weights [ci,co], rhs = shifted windows) producing
           [co, 4 rows * 114]; bias-add on the PSUM->SBUF copy; transpose
           back per row via TensorE; DMA out.
"""

import numpy as np

import concourse.bass as bass
import concourse.tile as tile
from concourse import bacc, mybir
from concourse import bass_utils
from concourse.masks import make_identity

F32 = mybir.dt.float32
BF16 = mybir.dt.bfloat16

N_CORES = 8
N_FULL = 64          # full batch
H = 112
W = 112
C = 64
EPS = 1e-5
BW = 128             # buffer row pitch (1 left pad + 112 px + 15 right pad)
NROW = H + 2         # 114 buffer rows (top/bottom pad rows)
XT_LEN = BW * NROW + 4   # +4: last window (r=111, dh=2, dw=2) overruns by 2
P1_CHUNK = 49        # phase-1 pixel tiles per DMA chunk


def build_kernel(n_imgs: int, n_cores: int):
    """Build and compile the per-core Bass program."""
    npix = n_imgs * H * W
    tot = N_FULL * H * W  # global pixel count for the batch statistics

    nc = bacc.Bacc(
        "TRN2", target_bir_lowering=False, debug=False, num_devices=n_cores
    )
    x = nc.dram_tensor("x", [npix, C], F32, kind="ExternalInput").ap()
    kern = nc.dram_tensor("kern", [9, C, C], F32, kind="ExternalInput").ap()
    beta = nc.dram_tensor("beta", [C, 1], F32, kind="ExternalInput").ap()
    out = nc.dram_tensor("out", [npix, C], F32, kind="ExternalOutput").ap()

    with tile.TileContext(nc) as tc:
        _body(tc, out, x, kern, beta, n_imgs, n_cores, npix, tot)
    nc.compile()
    return nc


def _body(tc, out, x, kern, beta, n_imgs, n_cores, npix, tot):
    nc = tc.nc
    P = 128

    with (
        tc.tile_pool(name="singles", bufs=1) as singles,
        tc.tile_pool(name="small", bufs=1) as small,
        tc.tile_pool(name="p1", bufs=4) as p1pool,
        tc.tile_pool(name="xt", bufs=2) as xtpool,
        tc.tile_pool(name="slab", bufs=3) as slabpool,
        tc.tile_pool(name="otb", bufs=3) as otbpool,
        tc.tile_pool(name="ps_stats", bufs=2, space="PSUM") as ps_stats,
        tc.tile_pool(name="ps_t", bufs=3, space="PSUM") as ps_t,
        tc.tile_pool(name="ps_o", bufs=2, space="PSUM") as ps_o,
        tc.tile_pool(name="ps_c", bufs=1, space="PSUM") as ps_c,
        tc.tile_pool(name="dram", bufs=2, space="DRAM") as dram,
    ):
        ident = singles.tile([P, P], F32)
        make_identity(nc, ident)

        # ---------------- Phase 1: local stats via TensorE ----------------
        # acc[:, 0] = sum_px x[px, ch]; acc[:, 1:65] = x^T x (diag = sumsq)
        acc = singles.tile([C, C + 1], F32)
        nc.vector.memset(acc, 0.0)

        a_tot = npix // P                       # pixel tiles of 128
        n_chunks = (a_tot + P1_CHUNK - 1) // P1_CHUNK
        xp = x.rearrange("(p a) c -> p a c", p=P)   # [128, a_tot, 64]
        for ci in range(n_chunks):
            a0 = ci * P1_CHUNK
            cw = min(P1_CHUNK, a_tot - a0)
            xt = p1pool.tile([P, P1_CHUNK, C + 1], F32, tag="p1")
            nc.vector.memset(xt[:, :cw, 0:1], 1.0)
            nc.sync.dma_start(out=xt[:, :cw, 1:], in_=xp[:, a0 : a0 + cw, :])
            ps = ps_stats.tile([C, C + 1], F32, tag="st")
            for j in range(cw):
                nc.tensor.matmul(
                    ps,
                    lhsT=xt[:, j, 1:],
                    rhs=xt[:, j, :],
                    start=(j == 0),
                    stop=(j == cw - 1),
                )
            nc.vector.tensor_add(acc, acc, ps)

        # sumsq = diag(x^T x) via identity mask + row reduce
        masked = small.tile([C, C], F32)
        nc.vector.tensor_mul(masked, acc[:, 1:], ident[:C, :C])
        loc = small.tile([C, 2], F32)
        nc.vector.tensor_copy(loc[:, 0:1], acc[:, 0:1])
        nc.vector.reduce_sum(loc[:, 1:2], masked, axis=mybir.AxisListType.X)

        # ---------------- AllReduce batch stats across cores ----------------
        cin = dram.tile([C, 2], F32)
        cout = dram.tile([C, 2], F32, addr_space="Shared")
        nc.sync.dma_start(out=cin, in_=loc)
        nc.gpsimd.collective_compute(
            "AllReduce",
            mybir.AluOpType.add,
            replica_groups=[list(range(n_cores))],
            ins=[cin[:].opt()],
            outs=[cout[:].opt()],
        )
        g = small.tile([C, 2], F32)
        nc.sync.dma_start(out=g, in_=cout)

        # ---------------- BN folding constants ----------------
        mean = small.tile([C, 1], F32)
        nc.vector.tensor_scalar_mul(mean, g[:, 0:1], 1.0 / tot)
        e2 = small.tile([C, 1], F32)
        nc.vector.tensor_scalar_mul(e2, g[:, 1:2], 1.0 / tot)
        msq = small.tile([C, 1], F32)
        nc.vector.tensor_mul(msq, mean, mean)
        var = small.tile([C, 1], F32)
        nc.vector.tensor_sub(var, e2, msq)
        eps_t = small.tile([C, 1], F32)
        nc.vector.memset(eps_t, EPS)
        std = small.tile([C, 1], F32)
        nc.scalar.activation(
            std, var, mybir.ActivationFunctionType.Sqrt, bias=eps_t, scale=1.0
        )
        s = small.tile([C, 1], F32)
        nc.vector.reciprocal(s, std)

        beta_sb = small.tile([C, 1], F32)
        nc.sync.dma_start(out=beta_sb, in_=beta)
        # data is stored pre-scaled (s*x); padding value s*mean - beta makes
        # the BN zero-padding exact, and c[co] = sum K.T @ (beta - s*mean)
        # restores the additive BN term uniformly.
        sm = small.tile([C, 1], F32)
        nc.vector.tensor_mul(sm, s, mean)
        padv = small.tile([C, 1], F32)
        nc.vector.tensor_sub(padv, sm, beta_sb)
        negpad = small.tile([C, 1], F32)
        nc.vector.tensor_sub(negpad, beta_sb, sm)

        # weights: wt fp32 (exact +/-1), wb bf16 (exact +/-1)
        wt = singles.tile([C, 9, C], F32)
        nc.sync.dma_start(out=wt, in_=kern.rearrange("t i o -> i t o"))
        wb = singles.tile([C, 9, C], BF16)
        nc.vector.tensor_copy(wb, wt)

        # output bias c[co] = sum_tap K[tap].T @ (beta - s*mean)
        cps = ps_c.tile([C, 1], F32, tag="c")
        for t9 in range(9):
            nc.tensor.matmul(
                cps, lhsT=wt[:, t9, :], rhs=negpad, start=(t9 == 0), stop=(t9 == 8)
            )
        cbias = small.tile([C, 1], F32)
        nc.vector.tensor_copy(cbias, cps)
        # replicate c to all partitions as a [128, 64] row-bias tile:
        # transpose [64,1] -> [1,64] on PE, bounce via DRAM with a
        # partition-broadcast access pattern.
        cpt = ps_c.tile([1, C], F32, tag="c")
        nc.tensor.matmul(cpt, lhsT=cbias, rhs=ident[:C, :C], start=True, stop=True)
        crow = small.tile([1, C], F32)
        nc.vector.tensor_copy(crow, cpt)
        crow_d = dram.tile([1, C], F32)
        nc.sync.dma_start(out=crow_d, in_=crow)
        cb128 = singles.tile([128, C], F32)
        nc.sync.dma_start(out=cb128, in_=crow_d[:].to_broadcast((128, C)))

        # bf16 identity for the input transposes
        identb = singles.tile([W, W], BF16)
        nc.vector.tensor_copy(identb, ident[:W, :W])

        # ---------------- Phase 2: conv per image ----------------
        # xT buffer: bf16, channel-major, pre-scaled by s. Row pitch 128:
        # buffer row j (input row j-1) at [128j, 128j+128) = [pad, 112 px,
        # 15 pad]. Conv: out(r, w) for one image row = 9 matmuls with
        # lhsT = xT[:, 128(r+dh)+dw : +128] (stationary, FWL-eligible) and
        # rhs = wb[tap] — PSUM comes out [w, co], already in NHWC order.
        x3 = x.rearrange("(r w) c -> r w c", w=W)    # [n_imgs*112, 112, 64]
        o3 = out.rearrange("(r w) c -> r w c", w=W)
        SLAB = 28                                    # rows per input DMA
        RG = 4                                       # out rows per DMA store

        for img in range(n_imgs):
            xtb = xtpool.tile([C, XT_LEN], BF16, tag="xt")
            xv = xtb[:, : BW * NROW].rearrange("p (j q) -> p j q", q=BW)
            # pads (value s*mean - beta per channel): top/bottom rows fully,
            # left col + right 15 cols of every row; +4 tail elements.
            for region in (
                xv[:, 0, :],
                xv[:, NROW - 1, :],
                xv[:, :, 0:1],
                xv[:, :, 1 + W :],
                xtb[:, BW * NROW :],
            ):
                nc.vector.memset(region, 0.0)
                nc.vector.tensor_scalar_add(region, region, padv)

            # rows -> channel-major (scaled, bf16) via regular-matmul transpose
            for sl in range(H // SLAB):
                slab = slabpool.tile([W, SLAB, C], BF16, tag="slab")
                r0 = img * H + sl * SLAB
                nc.gpsimd.dma_start(
                    out=slab,
                    in_=x3[r0 : r0 + SLAB, :, :].rearrange("r w c -> w r c"),
                )
                for rr in range(SLAB):
                    r = sl * SLAB + rr
                    pst = ps_t.tile([C, W], F32, tag="t")
                    nc.tensor.matmul(
                        pst, lhsT=slab[:, rr, :], rhs=identb,
                        start=True, stop=True,
                    )
                    dst0 = BW * (r + 1) + 1
                    nc.scalar.activation(
                        xtb[:, dst0 : dst0 + W],
                        pst,
                        mybir.ActivationFunctionType.Identity,
                        scale=s,
                    )

            # conv: per output row, 9 accumulating matmuls -> [w, co] PSUM
            for g4 in range(H // RG):
                otb = otbpool.tile([W, RG, C], F32, tag="otb")
                for rr in range(RG):
                    r = g4 * RG + rr
                    po = ps_o.tile([BW, C], F32, tag="o")
                    for t9 in range(9):
                        dh, dw = divmod(t9, 3)
                        off = BW * (r + dh) + dw
                        nc.tensor.matmul(
                            po,
                            lhsT=xtb[:, off : off + BW],
                            rhs=wb[:, t9, :],
                            start=(t9 == 0),
                            stop=(t9 == 8),
                        )
                    nc.vector.tensor_add(otb[:, rr, :], po[:W, :], cb128[:W, :])
                ro = img * H + g4 * RG
                nc.sync.dma_start(
                    out=o3[ro : ro + RG, :, :].rearrange("r w c -> w r c"),
                    in_=otb,
                )


_CACHE = {}


def _get_kernel(n_imgs, n_cores):
    key = (n_imgs, n_cores)
    if key not in _CACHE:
        _CACHE[key] = build_kernel(n_imgs, n_cores)
    return _CACHE[key]


def kernel(x, kernels, beta):
    """Full inputs -> full output. Shards batch over 8 NeuronCores."""
    n = x.shape[0]
    per = n // N_CORES
    npix = per * H * W
    nc = _get_kernel(per, N_CORES)

    kern9 = np.ascontiguousarray(kernels.reshape(9, C, C), dtype=np.float32)
    beta2 = np.ascontiguousarray(beta.reshape(C, 1), dtype=np.float32)
    in_maps = []
    for ci in range(N_CORES):
        xs = np.ascontiguousarray(
            x[ci * per : (ci + 1) * per].reshape(npix, C), dtype=np.float32
        )
        in_maps.append({"x": xs, "kern": kern9, "beta": beta2})

    res = bass_utils.run_bass_kernel_spmd(
        nc, in_maps, core_ids=list(range(N_CORES)), trace=TRACE
    )
    global LAST_RESULTS
    LAST_RESULTS = res
    outs = [
        res.results[ci]["out"].reshape(per, H, W, C) for ci in range(N_CORES)
    ]
    return np.concatenate(outs, axis=0)


TRACE = False
LAST_RESULTS = None



# revision 7
# speedup vs baseline: 2.3253x; 2.3253x over previous
"""Trainium2 Bass kernel for nn_Conv2DRand: batchnorm (training-mode, batch
stats) + 3x3 SAME conv, NHWC, f32.

Full computation:
    mean/var over (N,H,W) per channel; x_bn = (x-mean)*rsqrt(var+eps) + beta
    out = conv2d(x_bn, kernels, SAME, stride 1, NHWC x HWIO -> NHWC)

Sharding: data-parallel over batch across 8 cores (8 images each); batch
statistics via a tiny cross-core AllReduce of [sum, sumsq] per channel.

v2 design (single x pass, weight-folded BN, 2-taps-per-matmul conv):

  BN folding: out = conv(x_pad, K*s) + c with x stored RAW (bf16), s folded
  into the weights, pad value mean - beta*std (which makes BN zero-padding
  exact) and c[co] = sum_tap K.T @ (beta - s*mean).

  Layout: per image a channel-major buffer xtb2 [128, 58 blocks x 128 cols]:
  block B holds buffer row 2B-1 (channels on partitions 0-63) and buffer row
  2B (partitions 64-127) at the SAME columns; buffer row j = input row j-1,
  rows 0/113 are pad rows. Within a block: col 0 = left pad, cols 1..112 =
  pixels, col 113 = right pad. All 8 images stay SBUF-resident, so x is read
  from HBM exactly once.

  Phase A (per image): DMA slabs of 28 rows (f32->bf16 cast on gpsimd DMA) in
  pixel-major [112, chunk, 2x64]; per 2-row chunk one PE matmul against a
  112x112 identity transposes it to channel-major [128, 112] in PSUM; groups
  of 4 chunks are evacuated with one ScalarE copy into xtb2 and fed to one
  DVE bn_stats (pixels only -- pads excluded by construction). bn_aggr per
  image + small DVE math produce per-channel [sum, sumsq]; AllReduce; then
  fold s into the weights and fill the pad cells.

  Phase B conv (per image): for each block B and tap column dw, ONE stationary
  operand xtb2[:, B, dw:dw+112] (128-partition contraction = 2 buffer rows)
  feeds matmuls whose rhs stacks per-output-row weight pairs
  [K(dh_top); K(dh_bot)] for up to 4 consecutive output rows = 4 adjacent
  64-col PSUM slots (N up to 256). Output rows live in PSUM banks of 8
  (col = (r%8)*64, partitions = pixel w); each bank accumulates 6 blocks x 3
  dw of matmuls (start on first piece, stop on last), then one DVE add fuses
  the +c bias while evacuating to SBUF, and an 8-row DMA (alternating Sync /
  ScalarE queues) stores NHWC output.
"""

import numpy as np

import concourse.bass as bass
import concourse.tile as tile
from concourse import bacc, mybir
from concourse import bass_utils
from concourse.masks import make_identity

F32 = mybir.dt.float32
BF16 = mybir.dt.bfloat16

N_CORES = 8
N_FULL = 64          # full batch
H = 112
W = 112
C = 64
EPS = 1e-5
NBLK = 58            # col blocks per image (2 buffer rows each; 114 rows tot)
BW2 = 128            # cols per block
XCOLS = NBLK * BW2   # 7424 cols per image


def _conv_pieces():
    """Per block B: list of (ss0, n, bank, col0) matmul pieces; each piece
    covers output rows r = 2B-3+ss for ss in [ss0, ss0+n), all within one
    8-row PSUM bank, at cols [(r0%8)*64 ...)."""
    pieces = {}
    per_bank = {}
    for B in range(NBLK):
        rows = [2 * B - 3 + ss for ss in range(4)]
        valid = [r for r in rows if 0 <= r < H]
        runs = []
        cur = []
        for r in valid:
            if cur and (r == cur[-1] + 1) and (r // 8 == cur[0] // 8):
                cur.append(r)
            else:
                if cur:
                    runs.append(cur)
                cur = [r]
        if cur:
            runs.append(cur)
        plist = []
        for run in runs:
            r0 = run[0]
            g = r0 // 8
            plist.append((r0 - (2 * B - 3), len(run), g, (r0 % 8) * 64))
            per_bank[g] = per_bank.get(g, 0) + 3  # 3 dw emissions per piece
        pieces[B] = plist
    return pieces, per_bank


def build_kernel(n_imgs: int, n_cores: int):
    npix = n_imgs * H * W
    tot = N_FULL * H * W  # global pixel count for the batch statistics

    nc = bacc.Bacc(
        "TRN2", target_bir_lowering=False, debug=False, num_devices=n_cores
    )
    x = nc.dram_tensor("x", [npix, C], F32, kind="ExternalInput").ap()
    kern = nc.dram_tensor("kern", [9, C, C], F32, kind="ExternalInput").ap()
    beta = nc.dram_tensor("beta", [C, 1], F32, kind="ExternalInput").ap()
    out = nc.dram_tensor("out", [npix, C], F32, kind="ExternalOutput").ap()

    with tile.TileContext(nc) as tc:
        _body(tc, out, x, kern, beta, n_imgs, n_cores, tot)
    nc.compile()
    return nc


def _body(tc, out, x, kern, beta, n_imgs, n_cores, tot):
    nc = tc.nc
    x3 = x.rearrange("(r w) c -> r w c", w=W)    # [n_imgs*112, 112, 64]
    o3 = out.rearrange("(r w) c -> r w c", w=W)
    PIECES, PER_BANK = _conv_pieces()
    NG = H // 8  # 14 psum bank generations per image

    with (
        tc.tile_pool(name="singles", bufs=1) as singles,
        tc.tile_pool(name="small", bufs=1) as small,
        tc.tile_pool(name="slab", bufs=3) as slabpool,
        tc.tile_pool(name="otb", bufs=3) as otbpool,
        tc.tile_pool(name="ps_t", bufs=3, space="PSUM") as ps_t,
        tc.tile_pool(name="ps_o", bufs=3, space="PSUM") as ps_o,
        tc.tile_pool(name="ps_c", bufs=1, space="PSUM") as ps_c,
        tc.tile_pool(name="dram", bufs=2, space="DRAM") as dram,
    ):
        ident = singles.tile([128, 128], F32)
        make_identity(nc, ident)
        identb = singles.tile([W, W], BF16)
        nc.vector.tensor_copy(identb, ident[:W, :W])

        # all 8 images channel-major, SBUF-resident
        xtb2 = singles.tile([128, n_imgs, NBLK, BW2], BF16)
        # bn_stats 6-tuples per <=4-chunk group (4 groups per slab)
        bnsall = singles.tile([128, n_imgs, 16, 6], F32)

        # ---------------- Phase A: transpose + stats ----------------
        for img in range(n_imgs):
            for a in range(4):                     # slabs of 28 rows
                slab = slabpool.tile([W, 14, 2 * C], BF16, tag="slab")
                r0 = img * H + a * 28
                nc.gpsimd.dma_start(
                    out=slab.rearrange("w k (two c) -> w k two c", two=2),
                    in_=x3[r0 : r0 + 28, :, :].rearrange(
                        "(k two) w c -> w k two c", two=2
                    ),
                )
                for g0 in range(0, 14, 4):         # psum groups of <=4 chunks
                    gw = min(4, 14 - g0)
                    pst = ps_t.tile([128, gw, W], F32, tag="pst")
                    for j in range(gw):
                        nc.tensor.matmul(
                            pst[:, j, :],
                            lhsT=slab[:, g0 + j, :],
                            rhs=identb,
                            start=(j == 0),
                            stop=(j == gw - 1),
                        )
                    k0 = a * 14 + g0               # chunk idx = block-1
                    nc.scalar.activation(
                        xtb2[:, img, k0 + 1 : k0 + 1 + gw, 1 : 1 + W],
                        pst,
                        mybir.ActivationFunctionType.Identity,
                        scale=1.0,
                    )
                    gi = a * 4 + g0 // 4
                    nc.vector.bn_stats(
                        bnsall[:, img, gi, :],
                        pst.rearrange("p k w -> p (k w)"),
                    )

        # ---------------- stats: aggregate, AllReduce, BN constants --------
        bimg = small.tile([128, n_imgs, 2], F32)
        for img in range(n_imgs):
            nc.vector.bn_aggr(bimg[:, img, :], bnsall[:, img, :, :])
        n1 = float(56 * W)  # elements per partition per image
        # per-image sum & sumsq (as mean*n, (var+mean^2)*n), then reduce
        msq = small.tile([128, n_imgs, 1], F32)
        nc.vector.tensor_mul(msq, bimg[:, :, 0:1], bimg[:, :, 0:1])
        e2 = small.tile([128, n_imgs, 1], F32)
        nc.vector.tensor_add(e2, bimg[:, :, 1:2], msq)
        loc128 = small.tile([128, 2], F32)
        nc.vector.reduce_sum(
            loc128[:, 0:1], bimg[:, :, 0:1], axis=mybir.AxisListType.XY
        )
        nc.vector.reduce_sum(loc128[:, 1:2], e2, axis=mybir.AxisListType.XY)
        # merge partition halves (same channel, different row parity)
        topm = small.tile([C, 2], F32)
        nc.sync.dma_start(out=topm, in_=loc128[C : 2 * C, :])
        loc = small.tile([C, 2], F32)
        nc.vector.tensor_add(loc, loc128[:C, :], topm)
        nc.vector.tensor_scalar_mul(loc, loc, n1)

        cin = dram.tile([C, 2], F32)
        cout = dram.tile([C, 2], F32, addr_space="Shared")
        nc.sync.dma_start(out=cin, in_=loc)
        nc.gpsimd.collective_compute(
            "AllReduce",
            mybir.AluOpType.add,
            replica_groups=[list(range(n_cores))],
            ins=[cin[:].opt()],
            outs=[cout[:].opt()],
        )
        g = small.tile([C, 2], F32)
        nc.sync.dma_start(out=g, in_=cout)

        mean = small.tile([C, 1], F32)
        nc.vector.tensor_scalar_mul(mean, g[:, 0:1], 1.0 / tot)
        e2g = small.tile([C, 1], F32)
        nc.vector.tensor_scalar_mul(e2g, g[:, 1:2], 1.0 / tot)
        msqg = small.tile([C, 1], F32)
        nc.vector.tensor_mul(msqg, mean, mean)
        var = small.tile([C, 1], F32)
        nc.vector.tensor_sub(var, e2g, msqg)
        eps_t = small.tile([C, 1], F32)
        nc.vector.memset(eps_t, EPS)
        std = small.tile([C, 1], F32)
        nc.scalar.activation(
            std, var, mybir.ActivationFunctionType.Sqrt, bias=eps_t, scale=1.0
        )
        s = small.tile([C, 1], F32)
        nc.vector.reciprocal(s, std)

        beta_sb = small.tile([C, 1], F32)
        nc.sync.dma_start(out=beta_sb, in_=beta)
        # pad value mean - beta*std; bias input beta - s*mean
        bstd = small.tile([C, 1], F32)
        nc.vector.tensor_mul(bstd, beta_sb, std)
        padv = small.tile([C, 1], F32)
        nc.vector.tensor_sub(padv, mean, bstd)
        sm = small.tile([C, 1], F32)
        nc.vector.tensor_mul(sm, s, mean)
        negpad = small.tile([C, 1], F32)
        nc.vector.tensor_sub(negpad, beta_sb, sm)
        padv128 = small.tile([128, 1], F32)
        nc.vector.tensor_copy(padv128[:C, :], padv)
        nc.sync.dma_start(out=padv128[C:, :], in_=padv)

        # ---------------- weights: fold s, build wquad ----------------
        wtf = singles.tile([C, 9, C], F32)
        nc.sync.dma_start(out=wtf, in_=kern.rearrange("t i o -> i t o"))
        wts = singles.tile([C, 9, C], F32)
        nc.vector.tensor_scalar_mul(wts, wtf, s)
        ksb = singles.tile([C, 9, C], BF16)
        nc.vector.tensor_copy(ksb, wts)
        # wquad[:, dw, ss, :]: rhs slot for output row r=2B-3+ss:
        #   top (partitions 0-63, buffer row 2B-1): K(2-ss, dw), 0 if ss=3
        #   bottom (64-127, buffer row 2B):        K(3-ss, dw), 0 if ss=0
        wquad = singles.tile([128, 3, 4, C], BF16)
        nc.vector.memset(wquad, 0.0)
        for ss in range(4):
            if ss <= 2:
                nc.vector.tensor_copy(
                    wquad[:C, :, ss, :],
                    ksb[:, (2 - ss) * 3 : (2 - ss) * 3 + 3, :],
                )
            if ss >= 1:
                nc.sync.dma_start(
                    out=wquad[C:, :, ss, :],
                    in_=ksb[:, (3 - ss) * 3 : (3 - ss) * 3 + 3, :],
                )

        # output bias c[co] = sum_tap K[tap].T @ (beta - s*mean)
        cps = ps_c.tile([C, 1], F32, tag="c")
        for t9 in range(9):
            nc.tensor.matmul(
                cps, lhsT=wtf[:, t9, :], rhs=negpad, start=(t9 == 0), stop=(t9 == 8)
            )
        cbias = small.tile([C, 1], F32)
        nc.vector.tensor_copy(cbias, cps)
        cpt = ps_c.tile([1, C], F32, tag="c")
        nc.tensor.matmul(cpt, lhsT=cbias, rhs=ident[:C, :C], start=True, stop=True)
        crow = small.tile([1, C], F32)
        nc.vector.tensor_copy(crow, cpt)
        crow_d = dram.tile([1, C], F32)
        nc.sync.dma_start(out=crow_d, in_=crow)
        cb128 = singles.tile([128, C], F32)
        nc.sync.dma_start(out=cb128, in_=crow_d[:].to_broadcast((128, C)))
        cb512 = singles.tile([128, 8, C], F32)
        for k in range(8):
            nc.vector.tensor_copy(cb512[:, k, :], cb128)

        # ---------------- pads ----------------
        for img in range(n_imgs):
            xi = xtb2[:, img]
            nc.vector.memset(xi[:, 0, :], 0.0)          # block 0 (both halves)
            nc.vector.memset(xi[:, NBLK - 1, :], 0.0)   # block 57
            nc.vector.tensor_scalar_add(
                xi[C:, 0, 0:114], xi[C:, 0, 0:114], padv
            )  # buffer row 0 = top pad row
            nc.vector.tensor_scalar_add(
                xi[:C, NBLK - 1, 0:114], xi[:C, NBLK - 1, 0:114], padv
            )  # buffer row 113 = bottom pad row
            for col in (0, 113):                        # left/right pad cols
                reg = xi[:, 1 : NBLK - 1, col : col + 1]
                nc.vector.memset(reg, 0.0)
                nc.vector.tensor_scalar_add(reg, reg, padv128)

        # ---------------- Phase B: conv ----------------
        for img in range(n_imgs):
            xi = xtb2[:, img]
            po = {}
            remaining = dict(PER_BANK)
            started = set()
            for B in range(NBLK):
                for dw in range(3):
                    lhsT = xi[:, B, dw : dw + W]
                    for (ss0, n, gb, col0) in PIECES[B]:
                        if gb not in po:
                            po[gb] = ps_o.tile(
                                [128, 512], F32, tag="po", name="po"
                            )
                        first = gb not in started
                        started.add(gb)
                        remaining[gb] -= 1
                        last = remaining[gb] == 0
                        nc.tensor.matmul(
                            po[gb][:W, col0 : col0 + n * 64],
                            lhsT=lhsT,
                            rhs=wquad[:, dw, ss0 : ss0 + n, :],
                            start=first,
                            stop=last,
                            skip_group_check=True,
                        )
                        if last:
                            otb = otbpool.tile([W, 8, C], F32, tag="otb")
                            nc.vector.tensor_add(
                                otb, po[gb][:W, :], cb512[:W, :, :]
                            )
                            rb = img * H + 8 * gb
                            eng = nc.sync if (gb % 2 == 0) else nc.scalar
                            eng.dma_start(
                                out=o3[rb : rb + 8, :, :].rearrange(
                                    "r w c -> w r c"
                                ),
                                in_=otb,
                            )
                            del po[gb]


_CACHE = {}


def _get_kernel(n_imgs, n_cores):
    key = (n_imgs, n_cores)
    if key not in _CACHE:
        _CACHE[key] = build_kernel(n_imgs, n_cores)
    return _CACHE[key]


def kernel(x, kernels, beta):
    """Full inputs -> full output. Shards batch over 8 NeuronCores."""
    n = x.shape[0]
    per = n // N_CORES
    npix = per * H * W
    nc = _get_kernel(per, N_CORES)

    kern9 = np.ascontiguousarray(kernels.reshape(9, C, C), dtype=np.float32)
    beta2 = np.ascontiguousarray(beta.reshape(C, 1), dtype=np.float32)
    in_maps = []
    for ci in range(N_CORES):
        xs = np.ascontiguousarray(
            x[ci * per : (ci + 1) * per].reshape(npix, C), dtype=np.float32
        )
        in_maps.append({"x": xs, "kern": kern9, "beta": beta2})

    res = bass_utils.run_bass_kernel_spmd(
        nc, in_maps, core_ids=list(range(N_CORES)), trace=TRACE
    )
    global LAST_RESULTS
    LAST_RESULTS = res
    outs = [
        res.results[ci]["out"].reshape(per, H, W, C) for ci in range(N_CORES)
    ]
    return np.concatenate(outs, axis=0)


TRACE = False
LAST_RESULTS = None
